# revision 49
# baseline (speedup 1.0000x reference)
"""Bidirectional Mamba (MixerModel) Trainium2 kernel.

Sharding: data-parallel over batch. 8 batch elements -> 8 NeuronCores.
Each core runs the full 2-direction x 4-layer model for its batch element
(no collectives; the backward direction consumes a host-flipped copy of the
input, and the softmax attention pool is order-invariant so the backward
output never needs unflipping). Host stacks the per-core [64] outputs.

On-chip layout is feature-major: activations live as [feature, T] tiles so
the selective-scan recurrence h_t = dA_t * h_{t-1} + dBx_t maps onto the
DVE tensor_tensor_scan instruction (recurrence along the free axis, one
independent channel per partition, d_inner=128 on partitions, one scan per
state index s=0..15). Each layer streams T in chunks of 1024 to bound SBUF;
scan state crosses chunks via a [128,16] carry tile, the causal conv via a
3-column tail.

PE-base rules honored throughout: matmul operands must start at SBUF
partition 0/32/64, so xproj weights are host-padded to land B at rows 0:16,
C at 32:48 and dt_raw at 64:68 of one [68,T] tile, and row-broadcasts use
one-hot selector matrices as lhsT. The per-state sum y = sum_s C_s*h_s is
accumulated on the PE via identity-matmul into PSUM (start/stop groups)
instead of a DVE add chain.
"""

import numpy as np

D_MODEL = 64
N_LAYER = 4
D_INNER = 128
D_STATE = 16
D_CONV = 4
DT_RANK = 4
EPS = 1e-5
T = 2048
B = 8
NCORES = 8
TCH = 1024             # T streaming chunk
NCH = T // TCH
MM = 512               # max matmul free dim (one PSUM bank)


def _legalize_sync_waits(nc, mybir, maxw=1):
    """This container's walrus only accepts one sync-wait command per
    instruction (newer bass emits several, e.g. on the kernel-tail drain).
    Split excess waits onto preceding same-engine NOPs — semantically
    identical: the engine blocks on each wait in turn before the original
    instruction issues."""
    for blk in nc.m.functions[0].blocks:
        newlist, changed = [], False
        for inst in blk.instructions:
            si = inst.sync_info
            waits = list(si.on_wait) if si and si.on_wait else []
            if len(waits) > maxw:
                k = 0
                while len(waits) > maxw:
                    chunk, waits = waits[:maxw], waits[maxw:]
                    newlist.append(mybir.InstNoOp(
                        name=f"{inst.name}-waitsplit{k}", engine=inst.engine,
                        sync_info=mybir.SyncInfo(on_wait=chunk, on_update=[])))
                    k += 1
                inst.sync_info = mybir.SyncInfo(
                    on_wait=waits, on_update=list(si.on_update or []))
                changed = True
            newlist.append(inst)
        if changed:
            blk.instructions = newlist


def build_nc(legalize=True):
    import concourse.bass as bass
    import concourse.mybir as mybir
    import concourse.tile as tile
    from contextlib import ExitStack

    dt32 = mybir.dt.float32
    dt16 = mybir.dt.bfloat16
    Alu = mybir.AluOpType
    Act = mybir.ActivationFunctionType

    nc = bass.Bass("TRN2", target_bir_lowering=False, debug=False,
                   num_devices=NCORES)

    # ---- DRAM I/O ----------------------------------------------------
    def din(name, shape):
        return nc.dram_tensor(name, list(shape), dt32, kind="ExternalInput").ap()

    xin = din("xin", (2, D_MODEL, T))            # fwd + flipped input, feature-major
    in_wT = din("in_wT", (2, N_LAYER, D_MODEL, 2 * D_INNER))
    conv_w = din("conv_w", (2, N_LAYER, D_INNER, D_CONV))
    conv_b = din("conv_b", (2, N_LAYER, D_INNER, 1))
    xproj_wTp = din("xproj_wTp", (2, N_LAYER, D_INNER, 68))
    dt_wTp = din("dt_wTp", (2, N_LAYER, 68, D_INNER))
    dt_b = din("dt_b", (2, N_LAYER, D_INNER, 1))
    A_in = din("A", (2, N_LAYER, D_INNER, D_STATE))
    D_in = din("Dp", (2, N_LAYER, D_INNER, 1))
    out_wT = din("out_wT", (2, N_LAYER, D_INNER, D_MODEL))
    nw_in = din("nw", (2, N_LAYER, D_MODEL, 1))
    nb_in = din("nb", (2, N_LAYER, D_MODEL, 1))
    nf_w = din("nf_w", (D_MODEL, 1))
    nf_b = din("nf_b", (D_MODEL, 1))
    pool_wT = din("pool_wT", (2, D_MODEL, 1))
    ll_wT2 = din("ll_wT2", (2, D_MODEL, D_MODEL))
    ll_b = din("ll_b", (D_MODEL, 1))
    ident_in = nc.dram_tensor("ident", [D_INNER, D_INNER], dt16,
                              kind="ExternalInput").ap()
    selmat_in = nc.dram_tensor("selmat", [48, D_STATE * D_INNER], dt16,
                               kind="ExternalInput").ap()

    out_d = nc.dram_tensor("out", [D_MODEL, 1], dt32, kind="ExternalOutput").ap()

    with tile.TileContext(nc) as tc, ExitStack() as ctx:
        const = ctx.enter_context(tc.tile_pool(name="const", bufs=1))
        sb = ctx.enter_context(tc.tile_pool(name="sb", bufs=2))
        sc = ctx.enter_context(tc.tile_pool(name="sc", bufs=3))      # scan chunk tiles
        rows = ctx.enter_context(tc.tile_pool(name="rows", bufs=2))  # [1,*] rows
        pproj = ctx.enter_context(tc.tile_pool(name="pproj", bufs=1, space="PSUM"))
        pbc = ctx.enter_context(tc.tile_pool(name="pbc", bufs=4, space="PSUM"))
        py = ctx.enter_context(tc.tile_pool(name="py", bufs=1, space="PSUM"))

        # ---- constants (bf16 where they feed matmuls: 4x PE rate) -----
        ident = const.tile([D_INNER, D_INNER], dt16, tag="ident")
        nc.sync.dma_start(out=ident, in_=ident_in)
        # selmat[k, s*128+m] = (k==s): broadcast row s of a [16,N] block
        # across 128 output partitions via matmul (B block at base 0, C at 32)
        selmat = const.tile([48, D_STATE * D_INNER], dt16, tag="selmat")
        nc.sync.dma_start(out=selmat, in_=selmat_in)
        ones_row = const.tile([1, D_INNER], dt16, tag="ones_row")
        nc.vector.memset(ones_row, 1.0)
        # LN stats lhsT columns (scaled by 1/64); duplicated at base 64 so
        # the base-partition of lhsT matches rhs for direction 1 halves.
        lnsel = const.tile([D_INNER, 2], dt32, tag="lnsel")
        nc.vector.memset(lnsel, 1.0 / D_MODEL)
        eps_c = const.tile([1, 1], dt32, tag="epsc")
        nc.vector.memset(eps_c, EPS)
        one_col = const.tile([D_INNER, 1], dt32, tag="onecol")
        nc.vector.memset(one_col, 1.0)

        def cload(tag, ap_src, shape):
            t = const.tile(list(shape), dt32, tag=tag)
            nc.sync.dma_start(out=t, in_=ap_src)
            return t

        P = {}
        for d in range(2):
            for l in range(N_LAYER):
                k = (d, l)
                P[("in_wT",) + k] = cload(f"in_wT{d}{l}", in_wT[d, l], (D_MODEL, 2 * D_INNER))
                P[("conv_w",) + k] = cload(f"conv_w{d}{l}", conv_w[d, l], (D_INNER, D_CONV))
                P[("conv_b",) + k] = cload(f"conv_b{d}{l}", conv_b[d, l], (D_INNER, 1))
                P[("xproj_wTp",) + k] = cload(f"xproj{d}{l}", xproj_wTp[d, l], (D_INNER, 68))
                P[("dt_wTp",) + k] = cload(f"dtw{d}{l}", dt_wTp[d, l], (68, D_INNER))
                P[("dt_b",) + k] = cload(f"dt_b{d}{l}", dt_b[d, l], (D_INNER, 1))
                P[("A",) + k] = cload(f"A{d}{l}", A_in[d, l], (D_INNER, D_STATE))
                P[("Dp",) + k] = cload(f"Dp{d}{l}", D_in[d, l], (D_INNER, 1))
                P[("out_wT",) + k] = cload(f"out_wT{d}{l}", out_wT[d, l], (D_INNER, D_MODEL))
                P[("nw",) + k] = cload(f"nw{d}{l}", nw_in[d, l], (D_MODEL, 1))
                P[("nb",) + k] = cload(f"nb{d}{l}", nb_in[d, l], (D_MODEL, 1))
        nfw_sb = cload("nfw", nf_w, (D_MODEL, 1))
        nfb_sb = cload("nfb", nf_b, (D_MODEL, 1))
        pw_sb = [cload(f"pw{d}", pool_wT[d], (D_MODEL, 1)) for d in range(2)]
        llw_sb = [cload(f"llw{d}", ll_wT2[d], (D_MODEL, D_MODEL)) for d in range(2)]
        llb_sb = cload("llb", ll_b, (D_MODEL, 1))

        # residual stream: both directions packed into one [128, T] tile,
        # direction d in partitions d*64:(d+1)*64.
        resb = sb.tile([2 * D_MODEL, T], dt32, tag="res")
        for d in range(2):
            nc.sync.dma_start(out=resb[d * D_MODEL:(d + 1) * D_MODEL, :], in_=xin[d])

        # ---- layernorm (features on partitions) for one T-chunk -------
        # src: [64, TCH] AP at base 0 or 64; writes hln_c [64, TCH] (base 0)
        def ln_chunk(src, base, nw_c, nb_c, hln_c):
            lnsel_b = lnsel[base:base + D_MODEL]
            mean_sb = rows.tile([1, TCH], dt16, tag="mean")
            rstd_sb = rows.tile([1, TCH], dt16, tag="rstd")
            for j in range(TCH // MM):
                sj = slice(j * MM, (j + 1) * MM)
                sqc = sb.tile([D_MODEL, MM], dt32, tag="sqc")
                nc.scalar.activation(sqc, src[:, sj], Act.Square)
                pm = pproj.tile([1, MM], dt32, tag="pp")
                nc.tensor.matmul(pm, lnsel_b[:, 0:1], src[:, sj],
                                 start=True, stop=True)
                nc.scalar.activation(mean_sb[:, sj], pm, Act.Copy)
                pq = pproj.tile([1, MM], dt32, tag="pp")
                nc.tensor.matmul(pq, lnsel[0:D_MODEL, 0:1], sqc,
                                 start=True, stop=True)
                scr = rows.tile([1, MM], dt32, tag="scr")
                nc.scalar.activation(scr, mean_sb[:, sj], Act.Square)
                nc.vector.tensor_sub(scr, pq, scr)       # var = E[x^2] - mean^2
                nc.scalar.activation(scr, scr, Act.Sqrt, bias=eps_c)
                with nc.allow_low_precision("rstd in bf16 feeds 4x bf16 bcast matmul"):
                    nc.vector.reciprocal(rstd_sb[:, sj], scr)
            for j in range(TCH // MM):
                sj = slice(j * MM, (j + 1) * MM)
                mb = pbc.tile([D_MODEL, MM], dt32, tag="bc")
                rb = pbc.tile([D_MODEL, MM], dt32, tag="bc")
                nc.tensor.matmul(mb, ones_row[:, :D_MODEL], mean_sb[:, sj],
                                 start=True, stop=True)
                nc.tensor.matmul(rb, ones_row[:, :D_MODEL], rstd_sb[:, sj],
                                 start=True, stop=True)
                nc.vector.tensor_sub(hln_c[:, sj], src[:, sj], mb)
                # hln = ((src-mean)*nw) * rstd_b ; then += nb
                nc.vector.scalar_tensor_tensor(hln_c[:, sj], hln_c[:, sj], nw_c, rb,
                                               op0=Alu.mult, op1=Alu.mult)
                nc.scalar.activation(hln_c[:, sj], hln_c[:, sj], Act.Identity,
                                     bias=nb_c)

        # ---- one mamba layer chunk for direction d ---------------------
        lstate = {}

        def mamba_layer(d, l, res_old, res_new, c):
            base = d * D_MODEL
            src = res_old[base:base + D_MODEL, :]
            if c == 0:
                carry = sb.tile([D_INNER, D_STATE], dt32, tag=f"carry{d}",
                                name=f"carry{d}_{l}")
                ctail = sb.tile([D_INNER, D_CONV - 1], dt32, tag=f"ctail{d}",
                                name=f"ctail{d}_{l}")
                lstate[d] = (carry, ctail)
            carry, ctail = lstate[d]
            if True:
                s_ = slice(c * TCH, (c + 1) * TCH)
                hln = sb.tile([D_MODEL, TCH], dt32, tag="hln")
                ln_chunk(src[:, s_], base, P[("nw", d, l)], P[("nb", d, l)], hln)

                # in_proj x -> xpad[:, 3:], z -> zsilu
                wx = P[("in_wT", d, l)]
                xpad = sb.tile([D_INNER, D_CONV - 1 + TCH], dt32, tag="xpad")
                if c == 0:
                    nc.vector.memset(xpad[:, 0:D_CONV - 1], 0.0)
                else:
                    nc.vector.tensor_copy(xpad[:, 0:D_CONV - 1], ctail)
                zsilu = sb.tile([D_INNER, TCH], dt32, tag="zsilu")
                px = pproj.tile([D_INNER, TCH], dt32, tag="pp")
                for j in range(TCH // MM):
                    sj = slice(c * TCH + j * MM, c * TCH + (j + 1) * MM)
                    nc.tensor.matmul(px[:, j * MM:(j + 1) * MM], wx[:, 0:D_INNER],
                                     hln[:, j * MM:(j + 1) * MM], start=True, stop=True)
                nc.scalar.activation(xpad[:, D_CONV - 1:], px, Act.Copy)
                pz = pproj.tile([D_INNER, TCH], dt32, tag="pp")
                for j in range(TCH // MM):
                    nc.tensor.matmul(pz[:, j * MM:(j + 1) * MM], wx[:, D_INNER:],
                                     hln[:, j * MM:(j + 1) * MM], start=True, stop=True)
                # silu(z) = z * sigmoid(z)  (sim-compatible decomposition)
                nc.scalar.activation(zsilu, pz, Act.Sigmoid)
                nc.vector.tensor_mul(zsilu, zsilu, pz)
                if c < NCH - 1:
                    nc.vector.tensor_copy(ctail, xpad[:, TCH:TCH + D_CONV - 1])

                # causal depthwise conv (GPSIMD; this walrus lacks Pool STT,
                # so taps go mul-into-tmp + add)
                cw = P[("conv_w", d, l)]
                xact = sb.tile([D_INNER, TCH], dt32, tag="xact")
                nc.gpsimd.tensor_scalar(xact, xpad[:, 0:TCH], cw[:, 0:1],
                                        P[("conv_b", d, l)], op0=Alu.mult, op1=Alu.add)
                for jj in range(1, D_CONV):
                    ctmp = sc.tile([D_INNER, TCH], dt32, tag="ctmp")
                    nc.gpsimd.tensor_scalar(ctmp, xpad[:, jj:jj + TCH],
                                            cw[:, jj:jj + 1], None, op0=Alu.mult)
                    nc.gpsimd.tensor_add(xact, xact, ctmp)
                xsig = sc.tile([D_INNER, TCH], dt32, tag="da")
                nc.scalar.activation(xsig, xact, Act.Sigmoid)
                nc.gpsimd.tensor_mul(xact, xact, xsig)

                # xproj (padded): psum rows 0:16 B, 32:48 C, 64:68 dt_raw.
                # B/C evacuate to bf16 (feeds 4x-rate bf16 broadcast matmuls),
                # dt_raw stays f32.
                bcs = sb.tile([48, TCH], dt16, tag="bcs")
                dtr = sb.tile([68, TCH], dt32, tag="dtr")
                pd_ = pproj.tile([68, TCH], dt32, tag="pp")
                for j in range(TCH // MM):
                    nc.tensor.matmul(pd_[:, j * MM:(j + 1) * MM], P[("xproj_wTp", d, l)],
                                     xact[:, j * MM:(j + 1) * MM], start=True, stop=True)
                nc.scalar.activation(bcs, pd_[0:48], Act.Copy)
                nc.scalar.activation(dtr[64:68], pd_[64:68], Act.Copy)

                # dt = softplus(dt_wTp.T[64:68] @ dt_raw + dt_b)
                dts = sb.tile([D_INNER, TCH], dt32, tag="dts")
                pt = pproj.tile([D_INNER, TCH], dt32, tag="pp")
                for j in range(TCH // MM):
                    nc.tensor.matmul(pt[:, j * MM:(j + 1) * MM],
                                     P[("dt_wTp", d, l)][64:68, :],
                                     dtr[64:68, j * MM:(j + 1) * MM],
                                     start=True, stop=True)
                # softplus(x) = ln(1 + exp(x))  (sim-compatible decomposition)
                nc.scalar.activation(dts, pt, Act.Exp, bias=P[("dt_b", d, l)])
                nc.scalar.activation(dts, dts, Act.Ln, bias=one_col)

                # u = dt * x  (GPSIMD)
                u = sb.tile([D_INNER, TCH], dt32, tag="u")
                nc.gpsimd.tensor_mul(u, dts, xact)

                # ---- selective scan over this chunk ---------------------
                A_c = P[("A", d, l)]
                pyt = py.tile([D_INNER, TCH], dt32, tag="py")
                for s in range(D_STATE):
                    da = sc.tile([D_INNER, TCH], dt16, tag="da")
                    nc.scalar.activation(da, dts, Act.Exp, scale=A_c[:, s:s + 1])
                    dbx = sc.tile([D_INNER, TCH], dt16, tag="dbx")
                    selB = selmat[0:D_STATE, s * D_INNER:(s + 1) * D_INNER]
                    for j in range(TCH // MM):
                        sj = slice(j * MM, (j + 1) * MM)
                        bb = pbc.tile([D_INNER, MM], dt32, tag="bc")
                        nc.tensor.matmul(bb, selB, bcs[0:D_STATE, sj],
                                         start=True, stop=True)
                        nc.vector.tensor_mul(dbx[:, sj], u[:, sj], bb)
                    hs = sc.tile([D_INNER, TCH], dt16, tag="hs")
                    if c == 0:
                        nc.vector.tensor_tensor_scan(hs, da, dbx, 0.0,
                                                     op0=Alu.mult, op1=Alu.add)
                    else:
                        nc.vector.tensor_tensor_scan(hs, da, dbx, carry[:, s:s + 1],
                                                     op0=Alu.mult, op1=Alu.add)
                    if c < NCH - 1:
                        nc.vector.tensor_copy(carry[:, s:s + 1], hs[:, TCH - 1:TCH])
                    yt = sc.tile([D_INNER, TCH], dt16, tag="dbx")
                    selC = selmat[32:32 + D_STATE, s * D_INNER:(s + 1) * D_INNER]
                    for j in range(TCH // MM):
                        sj = slice(j * MM, (j + 1) * MM)
                        cb = pbc.tile([D_INNER, MM], dt32, tag="bc")
                        nc.tensor.matmul(cb, selC, bcs[32:32 + D_STATE, sj],
                                         start=True, stop=True)
                        nc.vector.tensor_mul(yt[:, sj], hs[:, sj], cb)
                    # y-sum over s accumulated on PE via identity matmul
                    for j in range(TCH // MM):
                        nc.tensor.matmul(pyt[:, j * MM:(j + 1) * MM], ident,
                                         yt[:, j * MM:(j + 1) * MM],
                                         start=(s == 0), stop=(s == D_STATE - 1))

                # y = (x*D + ys) * silu(z)
                yacc = sb.tile([D_INNER, TCH], dt32, tag="yacc")
                nc.vector.scalar_tensor_tensor(yacc, xact, P[("Dp", d, l)], pyt,
                                               op0=Alu.mult, op1=Alu.add)
                nc.vector.tensor_mul(yacc, yacc, zsilu)

                # out_proj + residual update
                po = pproj.tile([D_MODEL, TCH], dt32, tag="pp")
                for j in range(TCH // MM):
                    nc.tensor.matmul(po[:, j * MM:(j + 1) * MM], P[("out_wT", d, l)],
                                     yacc[:, j * MM:(j + 1) * MM], start=True, stop=True)
                nc.vector.tensor_add(res_new[base:base + D_MODEL, s_], po, src[:, s_])

        import os
        n_layers = int(os.environ.get("BK_LAYERS", N_LAYER))
        n_dirs = int(os.environ.get("BK_DIRS", 2))
        do_head = os.environ.get("BK_HEAD", "1") == "1"
        for l in range(n_layers):
            res_new = sb.tile([2 * D_MODEL, T], dt32, tag="res")
            for c in range(NCH):
                for d in range(n_dirs):
                    mamba_layer(d, l, resb, res_new, c)
            resb = res_new

        # ---- head: final LN, softmax pool over T, linear ---------------
        pooled = [None, None]
        for d in (range(2) if do_head else range(0)):
            base = d * D_MODEL
            hlnf = sb.tile([D_MODEL, T], dt32, tag="hlnf")
            for c in range(NCH):
                s_ = slice(c * TCH, (c + 1) * TCH)
                ln_chunk(resb[base:base + D_MODEL, s_], base, nfw_sb, nfb_sb,
                         hlnf[:, s_])
            logits = rows.tile([1, T], dt32, tag="logits")
            for c4 in range(T // MM):
                pl = pproj.tile([1, MM], dt32, tag="pp")
                nc.tensor.matmul(pl, pw_sb[d], hlnf[:, c4 * MM:(c4 + 1) * MM],
                                 start=True, stop=True)
                nc.scalar.activation(logits[:, c4 * MM:(c4 + 1) * MM], pl, Act.Copy)
            smalls = rows.tile([1, 4], dt32, tag="smalls")
            nc.vector.reduce_max(smalls[:, 0:1], logits, axis=mybir.AxisListType.X)
            nc.vector.tensor_scalar_mul(smalls[:, 1:2], smalls[:, 0:1], -1.0)
            nc.scalar.activation(logits, logits, Act.Exp, bias=smalls[:, 1:2])
            nc.vector.reduce_sum(smalls[:, 2:3], logits, axis=mybir.AxisListType.X)
            nc.vector.reciprocal(smalls[:, 3:4], smalls[:, 2:3])
            a16 = rows.tile([1, T], dt16, tag="a16")
            nc.vector.tensor_scalar(a16, logits, smalls[:, 3:4], None,
                                    op0=Alu.mult)
            pl_prev = None
            for c4 in range(T // MM):
                sj = slice(c4 * MM, (c4 + 1) * MM)
                ab = pbc.tile([D_MODEL, MM], dt32, tag="bc")
                nc.tensor.matmul(ab, ones_row[:, :D_MODEL], a16[:, sj],
                                 start=True, stop=True)
                scr = sb.tile([D_MODEL, MM], dt32, tag="poolscr")
                nc.vector.tensor_mul(scr, hlnf[:, sj], ab)
                pld = rows.tile([D_MODEL, 1], dt32, tag=f"pooled{d}")
                nc.vector.reduce_sum(pld, scr, axis=mybir.AxisListType.X)
                if pl_prev is not None:
                    nc.vector.tensor_add(pld, pld, pl_prev)
                pl_prev = pld
            pooled[d] = pl_prev
        if do_head:
            pout = pproj.tile([D_MODEL, 1], dt32, tag="pp")
            nc.tensor.matmul(pout, llw_sb[0], pooled[0], start=True, stop=False)
            nc.tensor.matmul(pout, llw_sb[1], pooled[1], start=False, stop=True)
            out_sb = rows.tile([D_MODEL, 1], dt32, tag="outsb")
            nc.scalar.activation(out_sb, pout, Act.Identity, bias=llb_sb)
            nc.sync.dma_start(out=out_d, in_=out_sb)
        else:
            out_sb = rows.tile([D_MODEL, 1], dt32, tag="outsb")
            nc.vector.tensor_copy(out_sb, resb[0:D_MODEL, 0:1])
            nc.sync.dma_start(out=out_d, in_=out_sb)

    if legalize:
        _legalize_sync_waits(nc, mybir)
    return nc


def _selmat():
    sel = np.zeros((48, D_STATE * D_INNER), np.float32)
    for s in range(D_STATE):
        sel[s, s * D_INNER:(s + 1) * D_INNER] = 1.0
        sel[32 + s, s * D_INNER:(s + 1) * D_INNER] = 1.0
    return sel


def prep_inputs(inputs):
    """Host-side prep: transposed weights as lhsT layouts, flipped input."""
    import ml_dtypes
    f = np.float32
    c = np.ascontiguousarray
    x = np.asarray(inputs["x"], f)               # [8, 64, 32, 64]
    xf = x.reshape(B, D_MODEL, T)                # feature-major [64, T]
    xb = xf[:, :, ::-1]
    xproj_wT = np.asarray(inputs["xproj_w"], f).transpose(0, 1, 3, 2)  # [2,4,128,36]
    xproj_pad = np.zeros((2, N_LAYER, D_INNER, 68), f)
    xproj_pad[..., 0:D_STATE] = xproj_wT[..., DT_RANK:DT_RANK + D_STATE]       # B
    xproj_pad[..., 32:32 + D_STATE] = xproj_wT[..., DT_RANK + D_STATE:]        # C
    xproj_pad[..., 64:68] = xproj_wT[..., 0:DT_RANK]                           # dt_raw
    dt_wT = np.asarray(inputs["dt_w"], f).transpose(0, 1, 3, 2)        # [2,4,4,128]
    dt_pad = np.zeros((2, N_LAYER, 68, D_INNER), f)
    dt_pad[:, :, 64:68, :] = dt_wT
    ll_wT = np.asarray(inputs["ll_w"], f).T                            # [128, 64]
    common = {
        "in_wT": c(np.asarray(inputs["in_w"], f).transpose(0, 1, 3, 2)),
        "conv_w": c(np.asarray(inputs["conv_w"], f)),
        "conv_b": c(np.asarray(inputs["conv_b"], f)[..., None]),
        "xproj_wTp": xproj_pad,
        "dt_wTp": dt_pad,
        "dt_b": c(np.asarray(inputs["dt_b"], f)[..., None]),
        "A": c(-np.exp(np.asarray(inputs["A_log"], f))),
        "Dp": c(np.asarray(inputs["D"], f)[..., None]),
        "out_wT": c(np.asarray(inputs["out_w"], f).transpose(0, 1, 3, 2)),
        "nw": c(np.asarray(inputs["nw"], f)[..., None]),
        "nb": c(np.asarray(inputs["nb"], f)[..., None]),
        "nf_w": c(np.asarray(inputs["nf_w"], f)[:, None]),
        "nf_b": c(np.asarray(inputs["nf_b"], f)[:, None]),
        "pool_wT": c(np.stack([np.asarray(inputs["fp_w"], f).T,
                               np.asarray(inputs["bp_w"], f).T])),
        "ll_wT2": c(np.stack([ll_wT[0:D_MODEL], ll_wT[D_MODEL:]])),
        "ll_b": c(np.asarray(inputs["ll_b"], f)[:, None]),
        "ident": np.eye(D_INNER, dtype=f).astype(ml_dtypes.bfloat16),
        "selmat": _selmat().astype(ml_dtypes.bfloat16),
    }
    in_maps = []
    for b in range(B):
        m = dict(common)
        m["xin"] = c(np.stack([xf[b], xb[b]]))
        in_maps.append(m)
    return in_maps


def kernel(**inputs):
    from concourse.bass_utils import run_bass_kernel_spmd
    in_maps = prep_inputs(inputs)
    nc = build_nc()
    res = run_bass_kernel_spmd(nc, in_maps, core_ids=list(range(NCORES)))
    out = np.stack([res.results[b]["out"][:, 0] for b in range(B)])
    return out.astype(np.float32)
